# revision 1
# baseline (speedup 1.0000x reference)
"""Channel-attention kernel for Trainium2, data-parallel over 8 NeuronCores.

Sharding: 8 shards = batch(2) x 4 horizontal strips of 56 rows (= 8 window
rows of ps=7). Each core receives its strip plus a 1-row halo on each side
(zero-padded at image edges) so the depthwise 3x3 conv is exact, computes
qkv -> dwconv -> windowed channel attention -> proj locally, and returns its
56-row output strip. Weights are replicated.
"""
import numpy as np
import jax
import jax.numpy as jnp
from functools import partial

B, C, H, W = 2, 384, 224, 224
HEADS, PS = 8, 7
D = C // HEADS                # 48
SH = 56                       # strip height (output rows per core)
NSTRIP = 4
NCORES = 8


def _l2norm(t):
    n = jnp.sqrt(jnp.sum(t * t, axis=-1, keepdims=True))
    return t / jnp.maximum(n, 1e-12)


def _strip_forward(x, qkv_w, dw_w, temperature, proj_w):
    # x: [C, SH+2, W] strip with halo rows
    qkv = jnp.einsum('chw,oc->ohw', x, qkv_w)                     # [3C, 58, W]
    qkv = jax.lax.conv_general_dilated(
        qkv[None], dw_w, window_strides=(1, 1),
        padding=((0, 0), (1, 1)),                                 # VALID in H (halo), SAME in W
        feature_group_count=3 * C,
        dimension_numbers=('NCHW', 'OIHW', 'NCHW'))[0]            # [3C, 56, W]
    q, k, v = jnp.split(qkv, 3, axis=0)

    nh, nw = SH // PS, W // PS                                    # 8, 32

    def to_win(t):
        t = t.reshape(HEADS, D, nh, PS, nw, PS)
        t = t.transpose(2, 4, 0, 1, 3, 5)
        return t.reshape(nh * nw, HEADS, D, PS * PS)

    q, k, v = to_win(q), to_win(k), to_win(v)
    q = _l2norm(q)
    k = _l2norm(k)

    attn = jnp.einsum('whds,whes->whde', q, k) * temperature      # [win, H, D, D]
    attn = jax.nn.softmax(attn, axis=-1)
    out = jnp.einsum('whde,whes->whds', attn, v)                  # [win, H, D, PS*PS]

    out = out.reshape(nh, nw, HEADS, D, PS, PS)
    out = out.transpose(2, 3, 0, 4, 1, 5).reshape(C, SH, W)
    return jnp.einsum('chw,oc->ohw', out, proj_w)                 # [C, SH, W]


_pforward = jax.pmap(_strip_forward, in_axes=(0, None, None, None, None))


def _make_shards(x):
    # x: [B, C, H, W] fp32 -> [8, C, SH+2, W] with 1-row halo each side
    xp = np.pad(x, ((0, 0), (0, 0), (1, 1), (0, 0)))
    shards = np.empty((NCORES, C, SH + 2, W), dtype=x.dtype)
    for b in range(B):
        for s in range(NSTRIP):
            r0 = s * SH          # in padded coords this is (row-1) of the strip
            shards[b * NSTRIP + s] = xp[b, :, r0:r0 + SH + 2, :]
    return shards


def kernel(x, qkv_w, dw_w, temperature, proj_w):
    x = np.asarray(x, dtype=np.float32)
    shards = _make_shards(x)
    res = _pforward(shards, jnp.asarray(qkv_w), jnp.asarray(dw_w),
                    jnp.asarray(temperature), jnp.asarray(proj_w))
    res = np.asarray(res)                                         # [8, C, SH, W]
    out = np.empty((B, C, H, W), dtype=np.float32)
    for b in range(B):
        for s in range(NSTRIP):
            out[b, :, s * SH:(s + 1) * SH, :] = res[b * NSTRIP + s]
    return out

